# revision 31
# baseline (speedup 1.0000x reference)
"""Cox partial-likelihood NegativeLogLikelihood loss on 8 Trainium2 cores.

reference:
    mask[i, j] = (y[j] <= y[i])                       # (N, N)
    num[j] = sum_i exp(r_i) * mask[i, j]
    den[j] = sum_i mask[i, j]
    loss = -sum_j e_j * (r_j - log(num_j / den_j)) / sum_j e_j + 0.01 * ||W||_F

Strategy: shard columns j across the 8 cores (each core owns 2048 columns).
The N x 2048 mask is materialized on-chip in [128, 2048] tiles and contracted
on the TensorEngine against lhsT = [exp_hi, exp_lo, 1, 0...] (bf16 Dekker
split, padded to 32 rows) into PSUM.

Perf structure:
  * y is re-encoded on the host as monotone bf16 codes (rank -> bf16 bit
    pattern + 0x2000), so comparisons are exact in bf16 and the DVE
    tensor_scalar(is_le) compare runs in the 4x perf mode (~0.74us per
    [128, 2048] tile vs ~1.1us for the f32 compare).
  * Row-side code copies carry a +quarter-ulp offset so code_i' > code_j
    strictly for i == j: the ScalarE Sign producer yields exactly +/-1 and
    all tie/diagonal corrections vanish.  DVE produces 93 tiles, ACT 35
    (sign-encoded with halved weights; ~2.0us/tile), balancing the engines.
  * Matmuls are column-tiled 4 ways (tile_position=(0, 32g)): four thin-M
    matmuls execute concurrently in disjoint 32-column strips of the PE
    array.  PSUM group g accumulates i-tiles 32g..32g+31 at partitions
    32g..32g+31 (rows 3..31 zero-padded so the epilogue can read PSUM
    full-width).
  * The sign-encoding corrections (+V_half into hi/lo rows, +NACT/2 per
    partition into den) are folded in by one extra matmul per jj chunk
    against an all-ones rhs -- no scalar broadcast round-trip.
  * Epilogue: PSUM -> SBUF copies chunked across scalar+vector, a selector
    matmul folds the 4 groups' rows into [num | den], 2 contiguous-dest
    scatter DMAs redistribute to [128, 16] pf layout, and each core emits
    [e_sum, w_ssq, t_sum]; the host unshard sums t over cores and applies
    -t/e + 0.01*sqrt(w_ssq).
"""
import numpy as np
import ml_dtypes
import orjson

import concourse.bass as bass
import concourse.tile as tile
import concourse.mybir as mybir
from concourse.bass_utils import run_bass_kernel_spmd

F32 = mybir.dt.float32
BF16 = mybir.dt.bfloat16

N = 16384
NCORES = 8
JSHARD = N // NCORES            # 2048 columns per core
NT = N // 128                   # 128 i-tiles of 128 rows
NG = 4                          # PE column-strip groups
NR = NT // NG                   # 32 i-tiles (rounds) per group
NJJ = JSHARD // 512             # 4 matmul column chunks per core
NACT = 35                       # ACT-produced tiles (sign-encoded)
DEN_ROW = float(NACT) / 2.0     # per-partition den correction row


def tile_of(g, r):
    return 32 * g + r


def is_act(g, r):
    # group 3 entirely, plus the tail rounds of group 2
    return g == 3 or (g == 2 and r >= NR - (NACT - 32))


ACT_TILES = [tile_of(g, r) for g in range(NG) for r in range(NR) if is_act(g, r)]

# ---------------------------------------------------------------------------
# Workaround for the installed walrus accepting at most ONE sync-wait command
# per TPB instruction: split multi-wait instructions into preceding
# single-wait EventSemaphore instructions on the same engine.
# ---------------------------------------------------------------------------


def _fix_bir_multiwait(bir_json: bytes) -> bytes:
    d = orjson.loads(bir_json)
    counter = 0
    for fn in d.get("functions", []):
        stack = list(fn.get("blocks", []))
        while stack:
            block = stack.pop()
            stack.extend(block.get("blocks", []))
            new_insts = []
            for inst in block.get("instructions", []):
                sync = inst.get("sync_info") or {}
                waits = sync.get("on_wait") or []
                if len(waits) > 1:
                    for w in waits[:-1]:
                        counter += 1
                        new_insts.append({
                            "debug": inst.get("debug", 0),
                            "engine": inst.get("engine"),
                            "ins": [],
                            "name": f"esw_fix_{counter}",
                            "opcode": "EventSemaphore",
                            "outs": [],
                            "sync_info": {"on_update": [], "on_wait": [w]},
                        })
                    sync["on_wait"] = [waits[-1]]
                new_insts.append(inst)
            block["instructions"] = new_insts
    return orjson.dumps(d)


_patched = False


def _install_bir_fix():
    global _patched
    if _patched:
        return
    _patched = True
    import concourse.bass_utils as bu
    import concourse.bass2jax as b2j

    orig = bu.compile_bir_kernel

    def patched(bir_json, tmpdir, neff_name="file.neff"):
        if isinstance(bir_json, str):
            bir_json = bir_json.encode()
        return orig(_fix_bir_multiwait(bir_json), tmpdir, neff_name)

    bu.compile_bir_kernel = patched
    b2j.compile_bir_kernel = patched


# ---------------------------------------------------------------------------
# Kernel build
# ---------------------------------------------------------------------------

def build_kernel() -> bass.Bass:
    nc = bass.Bass()
    Sign = mybir.ActivationFunctionType.Sign

    # j-side codes, host-broadcast to all 128 partitions, bf16
    yb_d = nc.dram_tensor("yb", [128, JSHARD], BF16, kind="ExternalInput")
    # misc: [ycol+d | rcol | r_pf | e_pf | e_f | w | scale_b | indh_b | sel]
    MISC_W = NT + NT + 16 + 16 + NT + 1024 + NT + NT + 2
    misc = nc.dram_tensor("misc", [128, MISC_W], F32, kind="ExternalInput")
    out = nc.dram_tensor("out", [1, 3], F32, kind="ExternalOutput")

    with tile.TileContext(nc) as tc:
        with (
            tc.tile_pool(name="const", bufs=1) as const,
            tc.tile_pool(name="masks", bufs=22) as masks,
            tc.tile_pool(name="psacc", bufs=1, space="PSUM") as psacc,
            tc.tile_pool(name="psaux", bufs=1, space="PSUM") as psaux,
        ):
            # ---- DVE-local init first (no input deps; overlaps the DMAs)
            lhsT = const.tile([128, 32, NT], BF16)
            nc.vector.memset(lhsT[:, :, :], 0.0)
            ones_col = const.tile([128, 1], F32)
            nc.vector.memset(ones_col, 1.0)

            # ---- critical-path loads
            yb = const.tile([128, JSHARD], BF16)
            nc.sync.dma_start(out=yb, in_=yb_d[:, :])
            misc_sb = const.tile([128, MISC_W], F32)
            nc.gpsimd.dma_start(out=misc_sb, in_=misc[:, :])
            o = 0
            ycol_sb = misc_sb[:, o:o + NT]; o += NT    # code_i + delta (f32)
            rcol_sb = misc_sb[:, o:o + NT]; o += NT    # risk_pred col-major
            rpf_sb = misc_sb[:, o:o + 16]; o += 16
            epf_f = misc_sb[:, o:o + 16]; o += 16
            e_f = misc_sb[:, o:o + NT]; o += NT
            w_sb = misc_sb[:, o:o + 1024]; o += 1024
            scale_b = misc_sb[:, o:o + NT]; o += NT    # 0.5 on ACT cols else 1
            indh_b = misc_sb[:, o:o + NT]; o += NT     # 0.5 on ACT cols else 0
            sel_sb = misc_sb[:, o:o + 2]; o += 2       # group-fold selectors

            # ---- lhsT rows 0..2 = scale * [exp_hi | exp_lo | ones], bf16
            exp_sb = const.tile([128, NT], F32)
            nc.scalar.activation(exp_sb, rcol_sb, mybir.ActivationFunctionType.Exp)
            nc.vector.tensor_copy(lhsT[:, 0, :], exp_sb)          # hi = bf16(exp)
            hi32 = const.tile([128, NT], F32)
            nc.vector.tensor_copy(hi32, lhsT[:, 0, :])            # back to f32
            lo32 = const.tile([128, NT], F32)
            nc.vector.tensor_sub(lo32, exp_sb, hi32)              # f32 residual
            nc.vector.tensor_mul(lhsT[:, 0, :], hi32, scale_b)    # exact in bf16
            nc.vector.tensor_mul(lhsT[:, 1, :], lo32, scale_b)
            nc.vector.tensor_copy(lhsT[:, 2, :], scale_b)

            # ---- main loop: mask tiles + column-tiled matmul accumulation
            # acc group g lives at partitions [32g, 32g+32), banks by jj chunk
            acc = psacc.tile([128, NJJ * 512], F32)

            # PE HAM warm-up: dummy matmuls into the acc region (the real
            # chains clear it via start=True)
            for k in range(4):
                nc.tensor.matmul(
                    acc[0:1, 0:256], ones_col, w_sb[:, 0:256],
                    start=True, stop=True, skip_group_check=True,
                )

            lhsT_va = const.tile([128, 32], BF16)
            vh = const.tile([128, NT], F32)
            vred = const.tile([128, 1], F32)
            vhi32 = const.tile([128, 1], F32)
            vlo = const.tile([128, 1], F32)
            ones_b = const.tile([128, 512], BF16)
            vec3 = const.tile([128, 3], F32)

            for r in range(NR):
                mt = {}
                for g in range(NG):
                    t = tile_of(g, r)
                    m = masks.tile([128, JSHARD], BF16)
                    mt[g] = m
                    if is_act(g, r):
                        nc.scalar.activation(
                            m, yb, Sign, bias=ycol_sb[:, t:t + 1], scale=-1.0,
                        )
                    else:
                        nc.vector.tensor_scalar(
                            out=m, in0=yb,
                            scalar1=ycol_sb[:, t:t + 1], scalar2=None,
                            op0=mybir.AluOpType.is_le,
                        )
                for jj in range(NJJ):
                    for g in range(NG):
                        t = tile_of(g, r)
                        nc.tensor.matmul(
                            acc[32 * g:32 * g + 32, 512 * jj:512 * (jj + 1)],
                            lhsT[:, :, t],
                            mt[g][:, 512 * jj:512 * (jj + 1)],
                            start=(r == 0),
                            stop=(r == NR - 1 and g != 0),
                            tile_position=(0, 32 * g),
                        )
                # deferred off-critical DVE prologue work (keeps the first
                # mask tiles at the head of the DVE queue)
                if r == 2:
                    nc.vector.memset(lhsT_va, 0.0)
                    nc.vector.tensor_mul(vh, exp_sb, indh_b)
                    nc.vector.tensor_reduce(
                        out=vred, in_=vh, axis=mybir.AxisListType.X,
                        op=mybir.AluOpType.add)
                elif r == 3:
                    nc.vector.tensor_copy(lhsT_va[:, 0:1], vred)  # vred_hi
                    nc.vector.tensor_copy(vhi32, lhsT_va[:, 0:1])
                    nc.vector.tensor_sub(vlo, vred, vhi32)
                    nc.vector.tensor_copy(lhsT_va[:, 1:2], vlo)   # vred_lo
                    nc.vector.memset(lhsT_va[:, 2:3], DEN_ROW)
                elif r == 4:
                    nc.vector.memset(ones_b, 1.0)
                    nc.vector.tensor_reduce(
                        out=vec3[:, 0:1], in_=e_f, axis=mybir.AxisListType.X,
                        op=mybir.AluOpType.add)

            # correction matmuls close out the group-0 accumulation chains:
            # add [V_half_hi, V_half_lo, NACT/2] (summed over partitions by
            # the PE) into the group-0 rows for every column
            for jj in range(NJJ):
                nc.tensor.matmul(
                    acc[0:32, 512 * jj:512 * (jj + 1)],
                    lhsT_va, ones_b,
                    start=False, stop=True, tile_position=(0, 0),
                )

            # ---- late ACT-side work that overlaps the epilogue
            w2d = const.tile([128, 1024], F32)
            nc.scalar.activation(
                w2d, w_sb, mybir.ActivationFunctionType.Square,
                accum_out=vec3[:, 1:2],
            )

            # ---- epilogue: PSUM -> SBUF staging (chunked, both engines),
            # selector matmul folds 4 groups' (hi+lo) and den rows into
            # [num | den], 2 contiguous-dest scatter DMAs into pf layout.
            # pf mapping: x_pf[p, c] = x_shard[16*p + c]
            nd_all = const.tile([128, NJJ * 512], F32)
            fold = psaux.tile([2, NJJ * 512], F32, name="fold")
            for jj in range(NJJ):
                eng = nc.scalar if jj % 2 == 0 else nc.vector
                if jj % 2 == 0:
                    nc.scalar.copy(nd_all[:, 512 * jj:512 * (jj + 1)],
                                   acc[:, 512 * jj:512 * (jj + 1)])
                else:
                    nc.vector.tensor_copy(nd_all[:, 512 * jj:512 * (jj + 1)],
                                          acc[:, 512 * jj:512 * (jj + 1)])
                nc.tensor.matmul(
                    fold[:, 512 * jj:512 * (jj + 1)],
                    sel_sb, nd_all[:, 512 * jj:512 * (jj + 1)],
                    start=True, stop=True, skip_group_check=True,
                )
            nd2 = const.tile([2, NJJ * 512], F32)
            nc.scalar.copy(nd2[:, 0:1024], fold[:, 0:1024])
            nc.vector.tensor_copy(nd2[:, 1024:2048], fold[:, 1024:2048])
            num_pf = const.tile([128, 16], F32)
            den_pf = const.tile([128, 16], F32)
            nc.sync.dma_start(out=num_pf, in_=nd2[0:1, :])
            nc.sync.dma_start(out=den_pf, in_=nd2[1:2, :])

            # ---- wide final math on [128, 16]
            lnn = const.tile([128, 16], F32)
            nc.scalar.activation(lnn, num_pf, mybir.ActivationFunctionType.Ln)
            lnd = const.tile([128, 16], F32)
            nc.scalar.activation(lnd, den_pf, mybir.ActivationFunctionType.Ln)
            s1 = const.tile([128, 16], F32)
            nc.vector.tensor_sub(s1, rpf_sb, lnn)
            s2 = const.tile([128, 16], F32)
            nc.vector.scalar_tensor_tensor(
                out=s2, in0=s1, scalar=1.0, in1=lnd,
                op0=mybir.AluOpType.mult, op1=mybir.AluOpType.add)
            s3 = const.tile([128, 16], F32)
            nc.vector.scalar_tensor_tensor(
                out=s3, in0=s2, scalar=1.0, in1=epf_f,
                op0=mybir.AluOpType.mult, op1=mybir.AluOpType.mult,
                accum_out=vec3[:, 2:3])

            # ---- cross-partition fold: [e_sum, w_ssq, t_sum] into one row
            # (reuses a slice of the fold tile; WAR deps order it after nd2)
            sums = fold[0:1, 0:3]
            nc.tensor.matmul(sums, ones_col, vec3[:, :], start=True,
                             stop=True, skip_group_check=True)
            res3 = const.tile([1, 3], F32)
            nc.vector.tensor_copy(res3, sums)
            nc.gpsimd.dma_start(out=out[:, :], in_=res3)

    return nc


_nc_cache = None


def _get_nc():
    global _nc_cache
    if _nc_cache is None:
        _install_bir_fix()
        _nc_cache = build_kernel()
    return _nc_cache


def make_in_maps(risk_pred, y, e, W):
    """Host-side sharding: slice/reshape/encode the full inputs per core."""
    yflat = y.reshape(-1)
    # monotone distinct bf16 codes: rank -> bf16 bit pattern (+0x2000 keeps
    # every code and its successor a normal number in [2^-63, 2^64], so
    # all pairwise differences are far from f32 under/overflow)
    order = np.argsort(yflat, kind="stable")
    ranks = np.empty(N, np.uint16)
    ranks[order] = np.arange(N, dtype=np.uint16)
    codes_u16 = (ranks + np.uint16(0x2000)).astype(np.uint16)
    codes_bf16 = codes_u16.view(ml_dtypes.bfloat16)
    codes_f32 = codes_bf16.astype(np.float32)
    nxt_f32 = (codes_u16 + np.uint16(1)).view(ml_dtypes.bfloat16).astype(np.float32)
    # row-side codes get +quarter-gap so the diagonal compare is strict (+1)
    ycol_delta = codes_f32 + 0.25 * (nxt_f32 - codes_f32)

    ycol = ycol_delta.reshape(NT, 128).T                     # [p, t]
    rcol = risk_pred.reshape(NT, 128).T.astype(np.float32)
    ef = e.astype(np.float32).reshape(NT, 128).T
    w_flat = W.reshape(128, 1024).astype(np.float32)
    act_mask = np.zeros(NT, np.float32)
    act_mask[ACT_TILES] = 1.0
    scale_b = np.tile(1.0 - 0.5 * act_mask, (128, 1)).astype(np.float32)
    indh_b = np.tile(0.5 * act_mask, (128, 1)).astype(np.float32)
    # group-fold selector: col 0 sums the hi+lo rows (p%32 in {0,1}) of the
    # 4 PSUM groups, col 1 sums the den rows (p%32 == 2)
    p = np.arange(128)
    sel = np.stack([((p % 32) <= 1), ((p % 32) == 2)], axis=1).astype(np.float32)

    in_maps = []
    for c in range(NCORES):
        j0 = c * JSHARD
        rsh = risk_pred.reshape(-1)[j0:j0 + JSHARD]
        esh = e.astype(np.float32).reshape(-1)[j0:j0 + JSHARD]
        r_pf = rsh.reshape(128, 16).astype(np.float32)
        e_pf = esh.reshape(128, 16)
        misc = np.ascontiguousarray(np.concatenate(
            [ycol, rcol, r_pf, e_pf, ef, w_flat, scale_b, indh_b, sel],
            axis=1), dtype=np.float32)
        yb = np.ascontiguousarray(
            np.broadcast_to(codes_bf16[j0:j0 + JSHARD], (128, JSHARD)))
        in_maps.append(dict(yb=yb, misc=misc))
    return in_maps


def kernel(risk_pred, y, e, W, **run_kwargs):
    nc = _get_nc()
    in_maps = make_in_maps(
        np.asarray(risk_pred, np.float32),
        np.asarray(y, np.float32),
        np.asarray(e, np.int32),
        np.asarray(W, np.float32),
    )
    result = run_bass_kernel_spmd(nc, in_maps, core_ids=list(range(NCORES)),
                                  **run_kwargs)
    # gather/unshard: t_sum adds across cores; e_sum and w_ssq are computed
    # from replicated inputs (identical on every core)
    t_total = np.float32(0.0)
    for r in result.results:
        t_total = np.float32(t_total + r["out"][0, 2])
    e_sum = np.float32(result.results[0]["out"][0, 0])
    w_ssq = np.float32(result.results[0]["out"][0, 1])
    total = np.float32(-t_total / e_sum + np.float32(0.01) * np.sqrt(w_ssq))
    kernel.last_result = result
    return np.asarray(total, np.float32)


# revision 36
# speedup vs baseline: 1.0040x; 1.0040x over previous
"""Cox partial-likelihood NegativeLogLikelihood loss on 8 Trainium2 cores.

reference:
    mask[i, j] = (y[j] <= y[i])                       # (N, N)
    num[j] = sum_i exp(r_i) * mask[i, j]
    den[j] = sum_i mask[i, j]
    loss = -sum_j e_j * (r_j - log(num_j / den_j)) / sum_j e_j + 0.01 * ||W||_F

Strategy: shard columns j across the 8 cores (each core owns 2048 columns).
The N x 2048 mask is materialized on-chip in [128, 2048] tiles and contracted
on the TensorEngine against lhsT = [exp_hi, exp_lo, 1, 0...] (bf16 Dekker
split, padded to 32 rows) into PSUM.

Perf structure:
  * y is re-encoded on the host as monotone bf16 codes (rank -> bf16 bit
    pattern + 0x2000), so comparisons are exact in bf16 and the DVE
    tensor_scalar(is_le) compare runs in the 4x perf mode (~0.74us per
    [128, 2048] tile vs ~1.1us for the f32 compare).
  * Row-side code copies carry a +quarter-ulp offset so code_i' > code_j
    strictly for i == j: the ScalarE Sign producer yields exactly +/-1 and
    all tie/diagonal corrections vanish.  DVE produces 93 tiles, ACT 35
    (sign-encoded with halved weights; ~2.0us/tile), balancing the engines.
  * Matmuls are column-tiled 4 ways (tile_position=(0, 32g)): four thin-M
    matmuls execute concurrently in disjoint 32-column strips of the PE
    array.  PSUM group g accumulates i-tiles 32g..32g+31 at partitions
    32g..32g+31 (rows 3..31 zero-padded so the epilogue can read PSUM
    full-width).
  * The sign-encoding corrections (+V_half into hi/lo rows, +NACT/2 per
    partition into den) are folded in by one extra matmul per jj chunk
    against an all-ones rhs -- no scalar broadcast round-trip.
  * Epilogue: PSUM -> SBUF copies chunked across scalar+vector, a selector
    matmul folds the 4 groups' rows into [num | den], 2 contiguous-dest
    scatter DMAs redistribute to [128, 16] pf layout, and each core emits
    [e_sum, w_ssq, t_sum]; the host unshard sums t over cores and applies
    -t/e + 0.01*sqrt(w_ssq).
"""
import numpy as np
import ml_dtypes
import orjson

import concourse.bass as bass
import concourse.tile as tile
import concourse.mybir as mybir
from concourse.bass_utils import run_bass_kernel_spmd

F32 = mybir.dt.float32
BF16 = mybir.dt.bfloat16

N = 16384
NCORES = 8
JSHARD = N // NCORES            # 2048 columns per core
NT = N // 128                   # 128 i-tiles of 128 rows
NG = 4                          # PE column-strip groups
NR = NT // NG                   # 32 i-tiles (rounds) per group
NJJ = JSHARD // 512             # 4 matmul column chunks per core
NACT = 35                       # ACT-produced tiles (sign-encoded)
DEN_ROW = float(NACT) / 2.0     # per-partition den correction row
ACT_EXTRA = {8, 16, 24}         # mid-stream rounds for ACT's group-2 tiles
                                # (never the tail rounds: the last rounds'
                                # tiles gate the epilogue)


def tile_of(g, r):
    return 32 * g + r


def is_act(g, r):
    return g == 3 or (g == 2 and r in ACT_EXTRA)


ACT_TILES = [tile_of(g, r) for g in range(NG) for r in range(NR) if is_act(g, r)]

# ---------------------------------------------------------------------------
# Workaround for the installed walrus accepting at most ONE sync-wait command
# per TPB instruction: split multi-wait instructions into preceding
# single-wait EventSemaphore instructions on the same engine.
# ---------------------------------------------------------------------------


def _fix_bir_multiwait(bir_json: bytes) -> bytes:
    d = orjson.loads(bir_json)
    counter = 0
    for fn in d.get("functions", []):
        stack = list(fn.get("blocks", []))
        while stack:
            block = stack.pop()
            stack.extend(block.get("blocks", []))
            new_insts = []
            for inst in block.get("instructions", []):
                sync = inst.get("sync_info") or {}
                waits = sync.get("on_wait") or []
                if len(waits) > 1:
                    for w in waits[:-1]:
                        counter += 1
                        new_insts.append({
                            "debug": inst.get("debug", 0),
                            "engine": inst.get("engine"),
                            "ins": [],
                            "name": f"esw_fix_{counter}",
                            "opcode": "EventSemaphore",
                            "outs": [],
                            "sync_info": {"on_update": [], "on_wait": [w]},
                        })
                    sync["on_wait"] = [waits[-1]]
                new_insts.append(inst)
            block["instructions"] = new_insts
    return orjson.dumps(d)


_patched = False


def _install_bir_fix():
    global _patched
    if _patched:
        return
    _patched = True
    import concourse.bass_utils as bu
    import concourse.bass2jax as b2j

    orig = bu.compile_bir_kernel

    def patched(bir_json, tmpdir, neff_name="file.neff"):
        if isinstance(bir_json, str):
            bir_json = bir_json.encode()
        return orig(_fix_bir_multiwait(bir_json), tmpdir, neff_name)

    bu.compile_bir_kernel = patched
    b2j.compile_bir_kernel = patched


# ---------------------------------------------------------------------------
# Kernel build
# ---------------------------------------------------------------------------

def build_kernel() -> bass.Bass:
    nc = bass.Bass()
    Sign = mybir.ActivationFunctionType.Sign

    # j-side codes, host-broadcast to all 128 partitions, bf16
    yb_d = nc.dram_tensor("yb", [128, JSHARD], BF16, kind="ExternalInput")
    # zero padding rows for lhsT (cheaper to DMA than to memset on DVE)
    lz_d = nc.dram_tensor("lz", [128, 32 * NT], BF16, kind="ExternalInput")
    # crit: the columns every producer needs early
    CRIT_W = NT + NT + NT + NT
    crit = nc.dram_tensor("crit", [128, CRIT_W], F32, kind="ExternalInput")
    # rest: [r_pf | e_pf | e_f | w | sel]
    REST_W = 16 + 16 + NT + 1024 + 2
    rest = nc.dram_tensor("rest", [128, REST_W], F32, kind="ExternalInput")
    out = nc.dram_tensor("out", [1, 3], F32, kind="ExternalOutput")

    with tile.TileContext(nc) as tc:
        with (
            tc.tile_pool(name="const", bufs=1) as const,
            tc.tile_pool(name="masks", bufs=22) as masks,
            tc.tile_pool(name="psacc", bufs=1, space="PSUM") as psacc,
            tc.tile_pool(name="psaux", bufs=1, space="PSUM") as psaux,
        ):
            # ---- DVE-local init first (no input deps; overlaps the DMAs)
            ones_col = const.tile([128, 1], F32)
            nc.vector.memset(ones_col, 1.0)

            # ---- critical-path loads
            yb = const.tile([128, JSHARD], BF16)
            nc.sync.dma_start(out=yb, in_=yb_d[:, :])
            lhsT = const.tile([128, 32, NT], BF16)
            nc.sync.dma_start(out=lhsT[:, :, :], in_=lz_d[:, :])
            crit_sb = const.tile([128, CRIT_W], F32)
            nc.gpsimd.dma_start(out=crit_sb, in_=crit[:, :])
            o = 0
            ycol_sb = crit_sb[:, o:o + NT]; o += NT    # code_i + delta (f32)
            rcol_sb = crit_sb[:, o:o + NT]; o += NT    # risk_pred col-major
            scale_b = crit_sb[:, o:o + NT]; o += NT    # 0.5 on ACT cols else 1
            indh_b = crit_sb[:, o:o + NT]; o += NT     # 0.5 on ACT cols else 0
            rest_sb = const.tile([128, REST_W], F32)
            nc.scalar.dma_start(out=rest_sb, in_=rest[:, :])
            o = 0
            rpf_sb = rest_sb[:, o:o + 16]; o += 16
            epf_f = rest_sb[:, o:o + 16]; o += 16
            e_f = rest_sb[:, o:o + NT]; o += NT
            w_sb = rest_sb[:, o:o + 1024]; o += 1024
            sel_sb = rest_sb[:, o:o + 2]; o += 2       # group-fold selectors

            exp_sb = const.tile([128, NT], F32)
            nc.scalar.activation(exp_sb, rcol_sb, mybir.ActivationFunctionType.Exp)

            # ---- main loop: mask tiles + column-tiled matmul accumulation
            # acc group g lives at partitions [32g, 32g+32), banks by jj chunk
            acc = psacc.tile([128, NJJ * 512], F32)

            lhsT_va = const.tile([128, 32], BF16)
            vh = const.tile([128, NT], F32)
            vred = const.tile([128, 1], F32)
            vhi32 = const.tile([128, 1], F32)
            vlo = const.tile([128, 1], F32)
            hi32 = const.tile([128, NT], F32)
            lo32 = const.tile([128, NT], F32)
            ones_b = const.tile([128, 512], BF16)
            vec3 = const.tile([128, 3], F32)

            for r in range(NR):
                mt = {}
                for g in range(NG):
                    t = tile_of(g, r)
                    m = masks.tile([128, JSHARD], BF16)
                    mt[g] = m
                    if is_act(g, r):
                        nc.scalar.activation(
                            m, yb, Sign, bias=ycol_sb[:, t:t + 1], scale=-1.0,
                        )
                    else:
                        nc.vector.tensor_scalar(
                            out=m, in0=yb,
                            scalar1=ycol_sb[:, t:t + 1], scalar2=None,
                            op0=mybir.AluOpType.is_le,
                        )
                if r == 0:
                    # lhsT rows 0..2 = scale * [exp_hi | exp_lo | ones], bf16
                    # (emitted after the first masks so they head the DVE
                    # queue, but before any matmul reads lhsT)
                    nc.vector.tensor_copy(lhsT[:, 0, :], exp_sb)   # bf16(exp)
                    nc.vector.tensor_copy(hi32, lhsT[:, 0, :])     # back to f32
                    nc.vector.tensor_sub(lo32, exp_sb, hi32)       # f32 residual
                    nc.vector.tensor_mul(lhsT[:, 0, :], hi32, scale_b)
                    nc.vector.tensor_mul(lhsT[:, 1, :], lo32, scale_b)
                    nc.vector.tensor_copy(lhsT[:, 2, :], scale_b)
                for g in range(NG):
                    t = tile_of(g, r)
                    for jj in range(NJJ):
                        nc.tensor.matmul(
                            acc[32 * g:32 * g + 32, 512 * jj:512 * (jj + 1)],
                            lhsT[:, :, t],
                            mt[g][:, 512 * jj:512 * (jj + 1)],
                            start=(r == 0),
                            stop=(r == NR - 1),
                            tile_position=(0, 32 * g),
                        )
                # deferred off-critical DVE prologue work (keeps the first
                # mask tiles at the head of the DVE queue)
                if r == 2:
                    nc.vector.memset(lhsT_va, 0.0)
                    nc.vector.tensor_mul(vh, exp_sb, indh_b)
                    nc.vector.tensor_reduce(
                        out=vred, in_=vh, axis=mybir.AxisListType.X,
                        op=mybir.AluOpType.add)
                elif r == 3:
                    nc.vector.tensor_copy(lhsT_va[:, 0:1], vred)  # vred_hi
                    nc.vector.tensor_copy(vhi32, lhsT_va[:, 0:1])
                    nc.vector.tensor_sub(vlo, vred, vhi32)
                    nc.vector.tensor_copy(lhsT_va[:, 1:2], vlo)   # vred_lo
                    nc.vector.memset(lhsT_va[:, 2:3], DEN_ROW)
                elif r == 4:
                    nc.vector.memset(ones_b, 1.0)
                elif r == 8:
                    nc.vector.tensor_reduce(
                        out=vec3[:, 0:1], in_=e_f, axis=mybir.AxisListType.X,
                        op=mybir.AluOpType.add)
                elif r == 6:
                    # correction matmuls, mid-stream: add [V_half_hi,
                    # V_half_lo, NACT/2] (summed over partitions by the PE)
                    # into the group-0 rows for every column
                    for jj in range(NJJ):
                        nc.tensor.matmul(
                            acc[0:32, 512 * jj:512 * (jj + 1)],
                            lhsT_va, ones_b,
                            start=False, stop=False, tile_position=(0, 0),
                        )

            # ---- late ACT-side work that overlaps the epilogue
            w2d = const.tile([128, 1024], F32)
            nc.scalar.activation(
                w2d, w_sb, mybir.ActivationFunctionType.Square,
                accum_out=vec3[:, 1:2],
            )

            # ---- epilogue: PSUM -> SBUF staging (chunked, both engines),
            # selector matmul folds 4 groups' (hi+lo) and den rows into
            # [num | den], 2 contiguous-dest scatter DMAs into pf layout.
            # pf mapping: x_pf[p, c] = x_shard[16*p + c]
            nd_all = const.tile([128, NJJ * 512], F32)
            fold = psaux.tile([2, NJJ * 512], F32, name="fold")
            for jj in range(NJJ):
                eng = nc.scalar if jj % 2 == 0 else nc.vector
                if jj % 2 == 0:
                    nc.scalar.copy(nd_all[:, 512 * jj:512 * (jj + 1)],
                                   acc[:, 512 * jj:512 * (jj + 1)])
                else:
                    nc.vector.tensor_copy(nd_all[:, 512 * jj:512 * (jj + 1)],
                                          acc[:, 512 * jj:512 * (jj + 1)])
                nc.tensor.matmul(
                    fold[:, 512 * jj:512 * (jj + 1)],
                    sel_sb, nd_all[:, 512 * jj:512 * (jj + 1)],
                    start=True, stop=True, skip_group_check=True,
                )
            nd2 = const.tile([2, NJJ * 512], F32)
            nc.scalar.copy(nd2[:, 0:1024], fold[:, 0:1024])
            nc.vector.tensor_copy(nd2[:, 1024:2048], fold[:, 1024:2048])
            num_pf = const.tile([128, 16], F32)
            den_pf = const.tile([128, 16], F32)
            nc.sync.dma_start(out=num_pf, in_=nd2[0:1, :])
            nc.sync.dma_start(out=den_pf, in_=nd2[1:2, :])

            # ---- wide final math on [128, 16]
            lnn = const.tile([128, 16], F32)
            nc.scalar.activation(lnn, num_pf, mybir.ActivationFunctionType.Ln)
            lnd = const.tile([128, 16], F32)
            nc.scalar.activation(lnd, den_pf, mybir.ActivationFunctionType.Ln)
            s1 = const.tile([128, 16], F32)
            nc.vector.tensor_sub(s1, rpf_sb, lnn)
            s2 = const.tile([128, 16], F32)
            nc.vector.scalar_tensor_tensor(
                out=s2, in0=s1, scalar=1.0, in1=lnd,
                op0=mybir.AluOpType.mult, op1=mybir.AluOpType.add)
            s3 = const.tile([128, 16], F32)
            nc.vector.scalar_tensor_tensor(
                out=s3, in0=s2, scalar=1.0, in1=epf_f,
                op0=mybir.AluOpType.mult, op1=mybir.AluOpType.mult,
                accum_out=vec3[:, 2:3])

            # ---- cross-partition fold: [e_sum, w_ssq, t_sum] into one row
            # (reuses a slice of the fold tile; WAR deps order it after nd2)
            sums = fold[0:1, 0:3]
            nc.tensor.matmul(sums, ones_col, vec3[:, :], start=True,
                             stop=True, skip_group_check=True)
            res3 = const.tile([1, 3], F32)
            nc.vector.tensor_copy(res3, sums)
            nc.gpsimd.dma_start(out=out[:, :], in_=res3)

    return nc


_nc_cache = None


def _get_nc():
    global _nc_cache
    if _nc_cache is None:
        _install_bir_fix()
        _nc_cache = build_kernel()
    return _nc_cache


def make_in_maps(risk_pred, y, e, W):
    """Host-side sharding: slice/reshape/encode the full inputs per core."""
    yflat = y.reshape(-1)
    # monotone distinct bf16 codes: rank -> bf16 bit pattern (+0x2000 keeps
    # every code and its successor a normal number in [2^-63, 2^64], so
    # all pairwise differences are far from f32 under/overflow)
    order = np.argsort(yflat, kind="stable")
    ranks = np.empty(N, np.uint16)
    ranks[order] = np.arange(N, dtype=np.uint16)
    codes_u16 = (ranks + np.uint16(0x2000)).astype(np.uint16)
    codes_bf16 = codes_u16.view(ml_dtypes.bfloat16)
    codes_f32 = codes_bf16.astype(np.float32)
    nxt_f32 = (codes_u16 + np.uint16(1)).view(ml_dtypes.bfloat16).astype(np.float32)
    # row-side codes get +quarter-gap so the diagonal compare is strict (+1)
    ycol_delta = codes_f32 + 0.25 * (nxt_f32 - codes_f32)

    ycol = ycol_delta.reshape(NT, 128).T                     # [p, t]
    rcol = risk_pred.reshape(NT, 128).T.astype(np.float32)
    ef = e.astype(np.float32).reshape(NT, 128).T
    w_flat = W.reshape(128, 1024).astype(np.float32)
    act_mask = np.zeros(NT, np.float32)
    act_mask[ACT_TILES] = 1.0
    scale_b = np.tile(1.0 - 0.5 * act_mask, (128, 1)).astype(np.float32)
    indh_b = np.tile(0.5 * act_mask, (128, 1)).astype(np.float32)
    # group-fold selector: col 0 sums the hi+lo rows (p%32 in {0,1}) of the
    # 4 PSUM groups, col 1 sums the den rows (p%32 == 2)
    p = np.arange(128)
    sel = np.stack([((p % 32) <= 1), ((p % 32) == 2)], axis=1).astype(np.float32)

    crit = np.ascontiguousarray(np.concatenate(
        [ycol, rcol, scale_b, indh_b], axis=1), dtype=np.float32)
    lz = np.zeros((128, 32 * NT), dtype=ml_dtypes.bfloat16)

    in_maps = []
    for c in range(NCORES):
        j0 = c * JSHARD
        rsh = risk_pred.reshape(-1)[j0:j0 + JSHARD]
        esh = e.astype(np.float32).reshape(-1)[j0:j0 + JSHARD]
        r_pf = rsh.reshape(128, 16).astype(np.float32)
        e_pf = esh.reshape(128, 16)
        rest = np.ascontiguousarray(np.concatenate(
            [r_pf, e_pf, ef, w_flat, sel], axis=1), dtype=np.float32)
        yb = np.ascontiguousarray(
            np.broadcast_to(codes_bf16[j0:j0 + JSHARD], (128, JSHARD)))
        in_maps.append(dict(yb=yb, lz=lz, crit=crit, rest=rest))
    return in_maps


def kernel(risk_pred, y, e, W, **run_kwargs):
    nc = _get_nc()
    in_maps = make_in_maps(
        np.asarray(risk_pred, np.float32),
        np.asarray(y, np.float32),
        np.asarray(e, np.int32),
        np.asarray(W, np.float32),
    )
    result = run_bass_kernel_spmd(nc, in_maps, core_ids=list(range(NCORES)),
                                  **run_kwargs)
    # gather/unshard: t_sum adds across cores; e_sum and w_ssq are computed
    # from replicated inputs (identical on every core)
    t_total = np.float32(0.0)
    for r in result.results:
        t_total = np.float32(t_total + r["out"][0, 2])
    e_sum = np.float32(result.results[0]["out"][0, 0])
    w_ssq = np.float32(result.results[0]["out"][0, 1])
    total = np.float32(-t_total / e_sum + np.float32(0.01) * np.sqrt(w_ssq))
    kernel.last_result = result
    return np.asarray(total, np.float32)


# revision 44
# speedup vs baseline: 1.0091x; 1.0050x over previous
"""Cox partial-likelihood NegativeLogLikelihood loss on 8 Trainium2 cores.

reference:
    mask[i, j] = (y[j] <= y[i])                       # (N, N)
    num[j] = sum_i exp(r_i) * mask[i, j]
    den[j] = sum_i mask[i, j]
    loss = -sum_j e_j * (r_j - log(num_j / den_j)) / sum_j e_j + 0.01 * ||W||_F

Strategy: shard columns j across the 8 cores (each core owns 2048 columns).
The N x 2048 mask is materialized on-chip in [128, 2048] tiles and contracted
on the TensorEngine against lhsT = [exp_hi, exp_lo, 1, 0...] (bf16 Dekker
split, padded to 32 rows) into PSUM.

Perf structure:
  * y is re-encoded on the host as monotone bf16 codes (rank -> bf16 bit
    pattern + 0x2000), so comparisons are exact in bf16 and the DVE
    tensor_scalar(is_le) compare runs in the 4x perf mode (~0.74us per
    [128, 2048] tile vs ~1.1us for the f32 compare).
  * Row-side code copies carry a +quarter-ulp offset so code_i' > code_j
    strictly for i == j: the ScalarE Sign producer yields exactly +/-1 and
    all tie/diagonal corrections vanish.  DVE produces 93 tiles, ACT 35
    (sign-encoded with halved weights; ~2.0us/tile), balancing the engines.
  * Matmuls are column-tiled 4 ways (tile_position=(0, 32g)): four thin-M
    matmuls execute concurrently in disjoint 32-column strips of the PE
    array.  PSUM group g accumulates i-tiles 32g..32g+31 at partitions
    32g..32g+31 (rows 3..31 zero-padded so the epilogue can read PSUM
    full-width).
  * The sign-encoding corrections (+V_half into hi/lo rows, +NACT/2 per
    partition into den) are folded in by one extra matmul per jj chunk
    against an all-ones rhs -- no scalar broadcast round-trip.
  * Epilogue: PSUM -> SBUF copies chunked across scalar+vector, a selector
    matmul folds the 4 groups' rows into [num | den], 2 contiguous-dest
    scatter DMAs redistribute to [128, 16] pf layout, and each core emits
    [e_sum, w_ssq, t_sum]; the host unshard sums t over cores and applies
    -t/e + 0.01*sqrt(w_ssq).
"""
import numpy as np
import ml_dtypes
import orjson

import concourse.bass as bass
import concourse.tile as tile
import concourse.mybir as mybir
from concourse.bass_utils import run_bass_kernel_spmd

F32 = mybir.dt.float32
BF16 = mybir.dt.bfloat16

N = 16384
NCORES = 8
JSHARD = N // NCORES            # 2048 columns per core
NT = N // 128                   # 128 i-tiles of 128 rows
NG = 4                          # PE column-strip groups
NR = NT // NG                   # 32 i-tiles (rounds) per group
NJJ = JSHARD // 512             # 4 matmul column chunks per core
NACT = 35                       # ACT-produced tiles (sign-encoded)
DEN_ROW = float(NACT) / 2.0     # per-partition den correction row


def tile_of(g, r):
    return 32 * g + r


def is_act(g, r):
    # ACT owns group 3 rounds 0..26 plus mid-stream extras in groups 1-2.
    # No ACT tiles in rounds >= 27: the serial ~1.9us/tile ACT stream would
    # otherwise gate the PE through the entire endgame.
    return ((g == 3 and r <= 26)
            or (g == 2 and r in (5, 10, 15, 20, 25))
            or (g == 1 and r in (7, 14, 21)))


ACT_TILES = [tile_of(g, r) for g in range(NG) for r in range(NR) if is_act(g, r)]

# ---------------------------------------------------------------------------
# Workaround for the installed walrus accepting at most ONE sync-wait command
# per TPB instruction: split multi-wait instructions into preceding
# single-wait EventSemaphore instructions on the same engine.
# ---------------------------------------------------------------------------


def _fix_bir_multiwait(bir_json: bytes) -> bytes:
    d = orjson.loads(bir_json)
    counter = 0
    for fn in d.get("functions", []):
        stack = list(fn.get("blocks", []))
        while stack:
            block = stack.pop()
            stack.extend(block.get("blocks", []))
            new_insts = []
            for inst in block.get("instructions", []):
                sync = inst.get("sync_info") or {}
                waits = sync.get("on_wait") or []
                if len(waits) > 1:
                    for w in waits[:-1]:
                        counter += 1
                        new_insts.append({
                            "debug": inst.get("debug", 0),
                            "engine": inst.get("engine"),
                            "ins": [],
                            "name": f"esw_fix_{counter}",
                            "opcode": "EventSemaphore",
                            "outs": [],
                            "sync_info": {"on_update": [], "on_wait": [w]},
                        })
                    sync["on_wait"] = [waits[-1]]
                new_insts.append(inst)
            block["instructions"] = new_insts
    return orjson.dumps(d)


_patched = False


def _install_bir_fix():
    global _patched
    if _patched:
        return
    _patched = True
    import concourse.bass_utils as bu
    import concourse.bass2jax as b2j

    orig = bu.compile_bir_kernel

    def patched(bir_json, tmpdir, neff_name="file.neff"):
        if isinstance(bir_json, str):
            bir_json = bir_json.encode()
        return orig(_fix_bir_multiwait(bir_json), tmpdir, neff_name)

    bu.compile_bir_kernel = patched
    b2j.compile_bir_kernel = patched


# ---------------------------------------------------------------------------
# Kernel build
# ---------------------------------------------------------------------------

def build_kernel() -> bass.Bass:
    nc = bass.Bass()
    Sign = mybir.ActivationFunctionType.Sign

    # j-side codes, host-broadcast to all 128 partitions, bf16
    yb_d = nc.dram_tensor("yb", [128, JSHARD], BF16, kind="ExternalInput")
    # crit: the columns every producer needs early
    CRIT_W = NT + NT + NT + NT
    crit = nc.dram_tensor("crit", [128, CRIT_W], F32, kind="ExternalInput")
    # rest: [r_pf | e_pf | e_f | w | sel]
    REST_W = 16 + 16 + NT + 1024 + 2
    rest = nc.dram_tensor("rest", [128, REST_W], F32, kind="ExternalInput")
    out = nc.dram_tensor("out", [1, 3], F32, kind="ExternalOutput")

    with tile.TileContext(nc) as tc:
        with (
            tc.tile_pool(name="const", bufs=1) as const,
            tc.tile_pool(name="masks", bufs=22) as masks,
            tc.tile_pool(name="psacc", bufs=1, space="PSUM") as psacc,
            tc.tile_pool(name="psaux", bufs=1, space="PSUM") as psaux,
        ):
            # ---- DVE-local init first (no input deps; overlaps the DMAs)
            ones_col = const.tile([128, 1], F32)
            nc.vector.memset(ones_col, 1.0)
            zeros_32 = const.tile([128, 32], BF16)
            nc.vector.memset(zeros_32, 0.0)
            lhsT = const.tile([128, 3, NT], BF16)

            # ---- critical-path loads (hw-dge queues: sync/scalar; the
            # gpsimd software-dge queue is ~10x slower per transfer)
            yb = const.tile([128, JSHARD], BF16)
            nc.sync.dma_start(out=yb, in_=yb_d[:, :])
            crit_sb = const.tile([128, CRIT_W], F32)
            nc.sync.dma_start(out=crit_sb, in_=crit[:, :])
            o = 0
            ycol_sb = crit_sb[:, o:o + NT]; o += NT    # code_i + delta (f32)
            rcol_sb = crit_sb[:, o:o + NT]; o += NT    # risk_pred col-major
            scale_b = crit_sb[:, o:o + NT]; o += NT    # 0.5 on ACT cols else 1
            indh_b = crit_sb[:, o:o + NT]; o += NT     # 0.5 on ACT cols else 0
            rest_sb = const.tile([128, REST_W], F32)
            nc.scalar.dma_start(out=rest_sb, in_=rest[:, :])
            o = 0
            rpf_sb = rest_sb[:, o:o + 16]; o += 16
            epf_f = rest_sb[:, o:o + 16]; o += 16
            e_f = rest_sb[:, o:o + NT]; o += NT
            w_sb = rest_sb[:, o:o + 1024]; o += 1024
            sel_sb = rest_sb[:, o:o + 2]; o += 2       # group-fold selectors

            exp_sb = const.tile([128, NT], F32)
            nc.scalar.activation(exp_sb, rcol_sb, mybir.ActivationFunctionType.Exp)

            # ---- main loop: mask tiles + column-tiled matmul accumulation
            # acc group g lives at partitions [32g, 32g+32), banks by jj chunk
            acc = psacc.tile([128, NJJ * 512], F32)

            lhsT_va = const.tile([128, 32], BF16)
            vh = const.tile([128, NT], F32)
            vred = const.tile([128, 1], F32)
            vhi32 = const.tile([128, 1], F32)
            vlo = const.tile([128, 1], F32)
            hi32 = const.tile([128, NT], F32)
            lo32 = const.tile([128, NT], F32)
            ones_b = const.tile([128, 512], BF16)
            vec3 = const.tile([128, 3], F32)

            for r in range(NR):
                mt = {}
                for g in range(NG):
                    t = tile_of(g, r)
                    m = masks.tile([128, JSHARD], BF16)
                    mt[g] = m
                    if is_act(g, r):
                        nc.scalar.activation(
                            m, yb, Sign, bias=ycol_sb[:, t:t + 1], scale=-1.0,
                        )
                    else:
                        nc.vector.tensor_scalar(
                            out=m, in0=yb,
                            scalar1=ycol_sb[:, t:t + 1], scalar2=None,
                            op0=mybir.AluOpType.is_le,
                        )
                if r == 0:
                    # lhsT rows 0..2 = scale * [exp_hi | exp_lo | ones], bf16
                    # (emitted after the first masks so they head the DVE
                    # queue, but before any matmul reads lhsT)
                    nc.vector.tensor_copy(lhsT[:, 0, :], exp_sb)   # bf16(exp)
                    nc.vector.tensor_copy(hi32, lhsT[:, 0, :])     # back to f32
                    nc.vector.tensor_sub(lo32, exp_sb, hi32)       # f32 residual
                    nc.vector.tensor_mul(lhsT[:, 0, :], hi32, scale_b)
                    nc.vector.tensor_mul(lhsT[:, 1, :], lo32, scale_b)
                    nc.vector.tensor_copy(lhsT[:, 2, :], scale_b)
                for g in range(NG):
                    t = tile_of(g, r)
                    for jj in range(NJJ):
                        nc.tensor.matmul(
                            acc[32 * g:32 * g + 3, 512 * jj:512 * (jj + 1)],
                            lhsT[:, :, t],
                            mt[g][:, 512 * jj:512 * (jj + 1)],
                            start=(r == 0),
                            stop=(r == NR - 1),
                            tile_position=(0, 32 * g),
                        )
                # deferred off-critical DVE prologue work (keeps the first
                # mask tiles at the head of the DVE queue)
                if r == 2:
                    nc.vector.memset(lhsT_va, 0.0)
                    nc.vector.tensor_mul(vh, exp_sb, indh_b)
                    nc.vector.tensor_reduce(
                        out=vred, in_=vh, axis=mybir.AxisListType.X,
                        op=mybir.AluOpType.add)
                elif r == 3:
                    nc.vector.tensor_copy(lhsT_va[:, 0:1], vred)  # vred_hi
                    nc.vector.tensor_copy(vhi32, lhsT_va[:, 0:1])
                    nc.vector.tensor_sub(vlo, vred, vhi32)
                    nc.vector.tensor_copy(lhsT_va[:, 1:2], vlo)   # vred_lo
                    nc.vector.memset(lhsT_va[:, 2:3], DEN_ROW)
                elif r == 4:
                    nc.vector.memset(ones_b, 1.0)
                elif r == 8:
                    nc.vector.tensor_reduce(
                        out=vec3[:, 0:1], in_=e_f, axis=mybir.AxisListType.X,
                        op=mybir.AluOpType.add)
                elif r == 6:
                    # correction matmuls, mid-stream, one [32, 512] matmul
                    # per (group, jj): group 0 adds [V_half_hi, V_half_lo,
                    # NACT/2] (summed over partitions by the PE) into its
                    # rows for every column; groups 1-3 use all-zero weights.
                    # All of them write rows 3..31 of their group (zeros via
                    # the zero-padded weight columns), initializing the PSUM
                    # partitions the main [3, 512] matmuls never touch so
                    # the epilogue can read PSUM full-width.
                    for g in range(NG):
                        w32 = lhsT_va if g == 0 else zeros_32
                        for jj in range(NJJ):
                            nc.tensor.matmul(
                                acc[32 * g:32 * g + 32,
                                    512 * jj:512 * (jj + 1)],
                                w32, ones_b,
                                start=False, stop=False,
                                tile_position=(0, 32 * g),
                                skip_group_check=True,
                            )

            # ---- late ACT-side work that overlaps the epilogue
            w2d = const.tile([128, 1024], F32)
            nc.scalar.activation(
                w2d, w_sb, mybir.ActivationFunctionType.Square,
                accum_out=vec3[:, 1:2],
            )

            # ---- epilogue: PSUM -> SBUF staging (chunked, both engines),
            # selector matmul folds 4 groups' (hi+lo) and den rows into
            # [num | den], 2 contiguous-dest scatter DMAs into pf layout.
            # pf mapping: x_pf[p, c] = x_shard[16*p + c]
            nd_all = const.tile([128, NJJ * 512], F32)
            fold = psaux.tile([2, NJJ * 512], F32, name="fold")
            for jj in range(NJJ):
                eng = nc.scalar if jj % 2 == 0 else nc.vector
                if jj % 2 == 0:
                    nc.scalar.copy(nd_all[:, 512 * jj:512 * (jj + 1)],
                                   acc[:, 512 * jj:512 * (jj + 1)])
                else:
                    nc.vector.tensor_copy(nd_all[:, 512 * jj:512 * (jj + 1)],
                                          acc[:, 512 * jj:512 * (jj + 1)])
                nc.tensor.matmul(
                    fold[:, 512 * jj:512 * (jj + 1)],
                    sel_sb, nd_all[:, 512 * jj:512 * (jj + 1)],
                    start=True, stop=True, skip_group_check=True,
                )
            nd2 = const.tile([2, NJJ * 512], F32)
            nc.scalar.copy(nd2[:, 0:1024], fold[:, 0:1024])
            nc.vector.tensor_copy(nd2[:, 1024:2048], fold[:, 1024:2048])
            num_pf = const.tile([128, 16], F32)
            den_pf = const.tile([128, 16], F32)
            nc.sync.dma_start(out=num_pf, in_=nd2[0:1, :])
            nc.scalar.dma_start(out=den_pf, in_=nd2[1:2, :])

            # ---- wide final math on [128, 16] (den first: its scatter is
            # on the scalar queue, so the Ln needs no cross-engine hop)
            lnd = const.tile([128, 16], F32)
            nc.scalar.activation(lnd, den_pf, mybir.ActivationFunctionType.Ln)
            lnn = const.tile([128, 16], F32)
            nc.scalar.activation(lnn, num_pf, mybir.ActivationFunctionType.Ln)
            s1 = const.tile([128, 16], F32)
            nc.vector.tensor_sub(s1, rpf_sb, lnn)
            s2 = const.tile([128, 16], F32)
            nc.vector.scalar_tensor_tensor(
                out=s2, in0=s1, scalar=1.0, in1=lnd,
                op0=mybir.AluOpType.mult, op1=mybir.AluOpType.add)
            s3 = const.tile([128, 16], F32)
            nc.vector.scalar_tensor_tensor(
                out=s3, in0=s2, scalar=1.0, in1=epf_f,
                op0=mybir.AluOpType.mult, op1=mybir.AluOpType.mult,
                accum_out=vec3[:, 2:3])

            # ---- cross-partition fold: [e_sum, w_ssq, t_sum] into one row
            # (reuses a slice of the fold tile; WAR deps order it after nd2)
            sums = fold[0:1, 0:3]
            nc.tensor.matmul(sums, ones_col, vec3[:, :], start=True,
                             stop=True, skip_group_check=True)
            res3 = const.tile([1, 3], F32)
            nc.vector.tensor_copy(res3, sums)
            nc.gpsimd.dma_start(out=out[:, :], in_=res3)

    return nc


_nc_cache = None


def _get_nc():
    global _nc_cache
    if _nc_cache is None:
        _install_bir_fix()
        _nc_cache = build_kernel()
    return _nc_cache


def make_in_maps(risk_pred, y, e, W):
    """Host-side sharding: slice/reshape/encode the full inputs per core."""
    yflat = y.reshape(-1)
    # monotone distinct bf16 codes: rank -> bf16 bit pattern (+0x2000 keeps
    # every code and its successor a normal number in [2^-63, 2^64], so
    # all pairwise differences are far from f32 under/overflow)
    order = np.argsort(yflat, kind="stable")
    ranks = np.empty(N, np.uint16)
    ranks[order] = np.arange(N, dtype=np.uint16)
    codes_u16 = (ranks + np.uint16(0x2000)).astype(np.uint16)
    codes_bf16 = codes_u16.view(ml_dtypes.bfloat16)
    codes_f32 = codes_bf16.astype(np.float32)
    nxt_f32 = (codes_u16 + np.uint16(1)).view(ml_dtypes.bfloat16).astype(np.float32)
    # row-side codes get +quarter-gap so the diagonal compare is strict (+1)
    ycol_delta = codes_f32 + 0.25 * (nxt_f32 - codes_f32)

    ycol = ycol_delta.reshape(NT, 128).T                     # [p, t]
    rcol = risk_pred.reshape(NT, 128).T.astype(np.float32)
    ef = e.astype(np.float32).reshape(NT, 128).T
    w_flat = W.reshape(128, 1024).astype(np.float32)
    act_mask = np.zeros(NT, np.float32)
    act_mask[ACT_TILES] = 1.0
    scale_b = np.tile(1.0 - 0.5 * act_mask, (128, 1)).astype(np.float32)
    indh_b = np.tile(0.5 * act_mask, (128, 1)).astype(np.float32)
    # group-fold selector: col 0 sums the hi+lo rows (p%32 in {0,1}) of the
    # 4 PSUM groups, col 1 sums the den rows (p%32 == 2)
    p = np.arange(128)
    sel = np.stack([((p % 32) <= 1), ((p % 32) == 2)], axis=1).astype(np.float32)

    crit = np.ascontiguousarray(np.concatenate(
        [ycol, rcol, scale_b, indh_b], axis=1), dtype=np.float32)

    in_maps = []
    for c in range(NCORES):
        j0 = c * JSHARD
        rsh = risk_pred.reshape(-1)[j0:j0 + JSHARD]
        esh = e.astype(np.float32).reshape(-1)[j0:j0 + JSHARD]
        r_pf = rsh.reshape(128, 16).astype(np.float32)
        e_pf = esh.reshape(128, 16)
        rest = np.ascontiguousarray(np.concatenate(
            [r_pf, e_pf, ef, w_flat, sel], axis=1), dtype=np.float32)
        yb = np.ascontiguousarray(
            np.broadcast_to(codes_bf16[j0:j0 + JSHARD], (128, JSHARD)))
        in_maps.append(dict(yb=yb, crit=crit, rest=rest))
    return in_maps


def kernel(risk_pred, y, e, W, **run_kwargs):
    nc = _get_nc()
    in_maps = make_in_maps(
        np.asarray(risk_pred, np.float32),
        np.asarray(y, np.float32),
        np.asarray(e, np.int32),
        np.asarray(W, np.float32),
    )
    result = run_bass_kernel_spmd(nc, in_maps, core_ids=list(range(NCORES)),
                                  **run_kwargs)
    # gather/unshard: t_sum adds across cores; e_sum and w_ssq are computed
    # from replicated inputs (identical on every core)
    t_total = np.float32(0.0)
    for r in result.results:
        t_total = np.float32(t_total + r["out"][0, 2])
    e_sum = np.float32(result.results[0]["out"][0, 0])
    w_ssq = np.float32(result.results[0]["out"][0, 1])
    total = np.float32(-t_total / e_sum + np.float32(0.01) * np.sqrt(w_ssq))
    kernel.last_result = result
    return np.asarray(total, np.float32)
